# revision 12
# baseline (speedup 1.0000x reference)
"""Deformable-correlation-fixed-weight kernel for 8 TRN2 NeuronCores.

Math: out[b, t*K+k, h, w] = sum_c samp[b,c,k,h,w] * weight[c,t,k].
With weight constant along c (DefCorFixW: weight = 1/C), this equals
s[t,k] * bilinear(mean_c x[b], py[b,k], px[b,k]); the device computes
the channel-mean image and the 9 bilinear-sampled maps per batch; the
host replicates over t and scales by s[t,k] = sum_c weight[c,t,k].

Sharding: data-parallel over batch B=8 across the 8 cores.

v5 design (multi-engine balance):
  Pixels n = h*96+w are laid out n = 72*p + i (p partition, i slot).
  Offsets clamped to +-3.9995 => a 9x9 hat window suffices.
  Taps k = 3*ky + kx processed as 3 triples of constant ky=tr.

  Engine split per triple:
    DVE:    3 per-tap window products + t4/t2 tree levels (2x mode),
            fused 3-tap t1/colred tails; y-stage only for triple 2.
    GPSIMD: dX/dY weight-grid builds for triples 1,2 and the y-stages
            of triples 0,1 (idle engine absorbs ~25us of DVE work).
    ACT:    PSUM mean drain (9x1024 copies) + all |.| and relu(1-.)
            hat evaluations over contiguous views.
    PE:     channel-mean matmuls chasing 3-chunk x DMAs.
  x arrives in 6 DMAs of 3 chunks (3KB packets, 2 per queue); the
  0.5MB delta table of v4 is replaced by tiny wtab/txw tables and
  on-device adds. Output in bf16, 3 per-triple DMAs.
"""

import numpy as np

B, C, H, W = 8, 128, 96, 96
K = 9
T = 9
HW = H * W
P = 128          # partitions
S = HW // P      # 72 pixels per partition
AW = 9           # window side (rows and cols)
CLAMP = 3.9995
PADR = 8         # zero rows above/below in the flat padded image
NPAD = (H + 2 * PADR) * W          # 10752
STRIPLEN = 1042                    # per-partition strip (flat span)
STRIPOFF = 283                     # 72p - 485 + PADR*96
NCH = 512
NCHUNK = HW // NCH                 # 18
TXW = S + 12                       # txw row stride (kx*(S+12) + s + j)
# impad sixth s may be written after ACT copy SIXTH_GATE[s] (1-based)
SIXTH_GATE = (2, 3, 5, 6, 8, 9)
# strip quarter q needs sC >= STRIP_GATE[q] (zt,zb,s0..s5 in order, 16 each)
STRIP_GATE = (64, 96, 112, 128)

_cached = {}


def _build_nc():
    import concourse.bass as bass
    import concourse.mybir as mybir
    from contextlib import ExitStack

    f32 = mybir.dt.float32
    bf16 = mybir.dt.bfloat16
    fp16 = mybir.dt.float16
    Alu = mybir.AluOpType
    Act = mybir.ActivationFunctionType

    nc = bass.Bass(detect_race_conditions=False)

    x_ext = nc.declare_dram_parameter("x", [C, HW], bf16, isOutput=False)
    off_ext = nc.declare_dram_parameter("offset", [P, 2 * K * S], fp16,
                                        isOutput=False)
    wtab_ext = nc.declare_dram_parameter("wtab", [P, S], fp16, isOutput=False)
    txw_ext = nc.declare_dram_parameter("txw", [P, 3 * TXW], fp16,
                                        isOutput=False)
    iotay_ext = nc.declare_dram_parameter("iotay", [P, AW], fp16,
                                          isOutput=False)
    ones_ext = nc.declare_dram_parameter("ones", [C, 2], bf16, isOutput=False)
    out_ext = nc.declare_dram_parameter("out", [P, K * S], bf16, isOutput=True)

    impad = nc.dram_tensor("impad", [NPAD], bf16)

    with ExitStack() as ctx:
        xb = ctx.enter_context(nc.sbuf_tensor([C, HW], bf16))
        off_sb = ctx.enter_context(nc.sbuf_tensor([P, 2 * K, S], fp16))
        wtab_sb = ctx.enter_context(nc.sbuf_tensor([P, S], fp16))
        txw_sb = ctx.enter_context(nc.sbuf_tensor([P, 3 * TXW], fp16))
        iotay_sb = ctx.enter_context(nc.sbuf_tensor([P, AW], fp16))
        ones_sb = ctx.enter_context(nc.sbuf_tensor([C, 2], bf16))
        m_flat = ctx.enter_context(nc.sbuf_tensor([1, HW], bf16))
        zt = ctx.enter_context(nc.sbuf_tensor([1, PADR * W], bf16))
        strip = ctx.enter_context(nc.sbuf_tensor([P, STRIPLEN], bf16))
        ox_cl = ctx.enter_context(nc.sbuf_tensor([P, K, S], fp16))
        oy_cl = ctx.enter_context(nc.sbuf_tensor([P, K, S], fp16))
        delta = ctx.enter_context(nc.sbuf_tensor([P, 3, S, AW], fp16))
        # dX3: t-major per triple [tr, t, s, j]; dY3: t-innermost [tr, s, j, t]
        dX3 = ctx.enter_context(nc.sbuf_tensor([P, 3, 3, S, AW], fp16))
        dY3 = ctx.enter_context(nc.sbuf_tensor([P, 3, S, AW, 3], fp16))
        aT = ctx.enter_context(nc.sbuf_tensor([P, S * AW * 3], fp16))
        wX3 = ctx.enter_context(nc.sbuf_tensor([P, 3, 3, S, AW], bf16))
        wY3 = ctx.enter_context(nc.sbuf_tensor([P, 3, S, AW, 3], bf16))
        prod = ctx.enter_context(nc.sbuf_tensor([P, 3, S, AW * AW], bf16))
        t4 = ctx.enter_context(nc.sbuf_tensor([P, 3, S, AW, 4], bf16))
        t2 = ctx.enter_context(nc.sbuf_tensor([P, 3, S, AW, 2], bf16))
        t1f = ctx.enter_context(nc.sbuf_tensor([P, 3 * S * AW], bf16))
        colredA = ctx.enter_context(nc.sbuf_tensor([P, 3, S * AW * 3], bf16))
        redG = ctx.enter_context(nc.sbuf_tensor([P, S, AW, 3], bf16))
        y4G = ctx.enter_context(nc.sbuf_tensor([P, S, 4, 3], bf16))
        y2G = ctx.enter_context(nc.sbuf_tensor([P, S, 2, 3], bf16))
        y1G = ctx.enter_context(nc.sbuf_tensor([P, S * 3], bf16))
        redV = ctx.enter_context(nc.sbuf_tensor([P, S, AW, 3], bf16))
        y4V = ctx.enter_context(nc.sbuf_tensor([P, S, 4, 3], bf16))
        y2V = ctx.enter_context(nc.sbuf_tensor([P, S, 2, 3], bf16))
        y1V = ctx.enter_context(nc.sbuf_tensor([P, S * 3], bf16))
        res = ctx.enter_context(nc.sbuf_tensor([P, 3, S * 3], bf16))
        psA = ctx.enter_context(nc.psum_tensor([2, 4096], f32))
        sIN = ctx.enter_context(nc.semaphore("sIN"))    # wtab+txw+iotay
        sI2 = ctx.enter_context(nc.semaphore("sI2"))    # ones
        sOF = ctx.enter_context(nc.semaphore("sOF"))    # offsets
        sC = ctx.enter_context(nc.semaphore("sC"))      # impad writes
        sD = ctx.enter_context(nc.semaphore("sD"))      # strip quarters
        sO = ctx.enter_context(nc.semaphore("sO"))      # out
        sXg = [ctx.enter_context(nc.semaphore(f"sXg{g}")) for g in range(6)]
        pe = ctx.enter_context(nc.semaphore("pe"))      # matmuls
        actC = ctx.enter_context(nc.semaphore("actC"))  # mean copies
        actX = ctx.enter_context(nc.semaphore("actX"))  # wX3 triples
        actY = ctx.enter_context(nc.semaphore("actY"))  # wY3 triples
        gp = ctx.enter_context(nc.semaphore("gp"))      # gpsimd milestones
        dve = ctx.enter_context(nc.semaphore("dve"))
        block = ctx.enter_context(nc.Block())

        # dve milestones: 1 memset, 2 delta, 3 ox, 4 oy, 5 dX0, 6 dY0,
        # 7/8/9 colred tr0/1/2, 10 y2 done
        # gp milestones: 1 dX1, 2 dX2, 3 dY1, 4 dY2, 5 y0 done, 6 y1 done
        def pap(t_ap, off, dims):
            return bass.AP(tensor=t_ap.tensor, offset=t_ap.offset + off,
                           ap=[list(t_ap.ap[0])] + dims)

        def xg_dma(eng, g):
            eng.dma_start(
                out=xb[:, g * 3 * NCH:(g + 1) * 3 * NCH],
                in_=x_ext[:, g * 3 * NCH:(g + 1) * 3 * NCH]).then_inc(
                    sXg[g], 16)

        @block.sync
        def _(sync):
            xg_dma(sync, 0)
            sync.dma_start(out=wtab_sb[:], in_=wtab_ext[:]).then_inc(sIN, 16)
            sync.dma_start(out=txw_sb[:], in_=txw_ext[:]).then_inc(sIN, 16)
            sync.dma_start(out=iotay_sb[:], in_=iotay_ext[:]).then_inc(sIN, 16)
            sync.dma_start(out=ones_sb[:], in_=ones_ext[:]).then_inc(sI2, 16)
            xg_dma(sync, 1)
            sync.wait_ge(dve, 1)
            sync.dma_start(
                out=bass.AP(tensor=impad[:].tensor, offset=impad[:].offset,
                            ap=[[1, 1], [1, PADR * W]]),
                in_=zt[:]).then_inc(sC, 16)
            sync.dma_start(
                out=bass.AP(tensor=impad[:].tensor,
                            offset=impad[:].offset + NPAD - PADR * W,
                            ap=[[1, 1], [1, PADR * W]]),
                in_=zt[:]).then_inc(sC, 16)
            sixth = HW // 6
            for s6 in range(6):
                sync.wait_ge(actC, SIXTH_GATE[s6])
                sync.dma_start(
                    out=bass.AP(tensor=impad[:].tensor,
                                offset=impad[:].offset + PADR * W + s6 * sixth,
                                ap=[[1, 1], [1, sixth]]),
                    in_=m_flat[:, s6 * sixth:(s6 + 1) * sixth]).then_inc(sC, 16)
            for q, gate in ((0, STRIP_GATE[0]), (2, STRIP_GATE[2]),
                            (3, STRIP_GATE[3])):
                sync.wait_ge(sC, gate)
                sync.dma_start(
                    out=strip[32 * q:32 * (q + 1)],
                    in_=bass.AP(tensor=impad[:].tensor,
                                offset=impad[:].offset + STRIPOFF + 32 * q * S,
                                ap=[[S, 32], [1, STRIPLEN]])).then_inc(sD, 16)
            sync.wait_ge(gp, 5)
            sync.dma_start(out=out_ext[:, 0:3 * S],
                           in_=res[:, 0]).then_inc(sO, 16)
            sync.wait_ge(gp, 6)
            sync.dma_start(out=out_ext[:, 3 * S:6 * S],
                           in_=res[:, 1]).then_inc(sO, 16)
            sync.wait_ge(dve, 10)
            sync.dma_start(out=out_ext[:, 6 * S:],
                           in_=res[:, 2]).then_inc(sO, 16)

        @block.gpsimd
        def _(g):
            g.dma_start(
                out=off_sb[:].rearrange("p a b -> p (a b)"),
                in_=off_ext[:]).then_inc(sOF, 16)
            xg_dma(g, 4)
            xg_dma(g, 5)
            g.wait_ge(sC, STRIP_GATE[1])
            g.dma_start(
                out=strip[32:64],
                in_=bass.AP(tensor=impad[:].tensor,
                            offset=impad[:].offset + STRIPOFF + 32 * S,
                            ap=[[S, 32], [1, STRIPLEN]])).then_inc(sD, 16)
            # weight grids for triples 1,2 (px0/txw from DVE/tables)
            g.wait_ge(dve, 3)
            for tr in (1, 2):
                # dX3[p,tr,t,s,j] = ox_cl[p,3tr+t,s] + delta[p,t,s,j]
                nc.gpsimd.tensor_tensor(
                    dX3[:, tr],
                    pap(ox_cl[:], 3 * tr * S, [[S, 3], [1, S], [0, AW]]),
                    pap(delta[:], 0, [[S * AW, 3], [AW, S], [1, AW]]),
                    Alu.add).then_inc(gp, 1)
            g.wait_ge(dve, 4)
            for tr in (1, 2):
                # dY3[p,tr,s,j,t] = oy_cl[p,3tr+t,s] - iotay[p,j]
                nc.gpsimd.tensor_tensor(
                    dY3[:, tr],
                    pap(oy_cl[:], 3 * tr * S, [[1, S], [0, AW], [S, 3]]),
                    pap(iotay_sb[:], 0, [[0, S], [1, AW], [0, 3]]),
                    Alu.subtract).then_inc(gp, 1)
            # y-stages for triples 0,1
            for tr in (0, 1):
                g.wait_ge(dve, 7 + tr)
                g.wait_ge(actY, tr + 1)
                nc.gpsimd.tensor_tensor(
                    redG[:].rearrange("p a b c -> p (a b c)"),
                    colredA[:, tr],
                    wY3[:, tr].rearrange("p a b c -> p (a b c)"),
                    Alu.mult)
                nc.gpsimd.tensor_tensor(
                    y4G[:],
                    pap(redG[:], 0, [[27, S], [3, 4], [1, 3]]),
                    pap(redG[:], 12, [[27, S], [3, 4], [1, 3]]),
                    Alu.add)
                nc.gpsimd.tensor_tensor(
                    y2G[:],
                    pap(y4G[:], 0, [[12, S], [3, 2], [1, 3]]),
                    pap(y4G[:], 6, [[12, S], [3, 2], [1, 3]]),
                    Alu.add)
                nc.gpsimd.tensor_tensor(
                    y1G[:],
                    pap(y2G[:], 0, [[6, S], [1, 3]]),
                    pap(y2G[:], 3, [[6, S], [1, 3]]),
                    Alu.add)
                nc.gpsimd.tensor_tensor(
                    res[:, tr], y1G[:],
                    pap(redG[:], 24, [[27, S], [1, 3]]),
                    Alu.add).then_inc(gp, 1)

        @block.tensor
        def _(tensor):
            tensor.wait_ge(sI2, 16)   # ones
            for g in range(NCHUNK):
                if g % 3 == 0:
                    tensor.wait_ge(sXg[g // 3], 16)
                if g >= 8:
                    tensor.wait_ge(actC, (g - 8) // 2 + 1)
                nc.tensor.matmul(
                    psA[:, (g % 8) * NCH:(g % 8 + 1) * NCH],
                    ones_sb[:],
                    xb[:, g * NCH:(g + 1) * NCH],
                    start=True, stop=True,
                ).then_inc(pe, 1)

        @block.scalar
        def _(scalar):
            xg_dma(scalar, 2)
            xg_dma(scalar, 3)
            for c in range(9):
                scalar.wait_ge(pe, 2 * c + 2)
                nc.scalar.activation(
                    m_flat[:, c * 1024:(c + 1) * 1024],
                    psA[0:1, (2 * c % 8) * NCH:(2 * c % 8 + 2) * NCH],
                    Act.Copy,
                ).then_inc(actC, 1)

            def hats(w_out, d_in, inc_sem):
                nc.scalar.activation(aT[:], d_in, Act.Abs)
                nc.scalar.activation(w_out, aT[:], Act.Relu,
                                     bias=1.0, scale=-1.0).then_inc(inc_sem, 1)

            scalar.wait_ge(dve, 5)
            hats(wX3[:, 0].rearrange("p a b c -> p (a b c)"),
                 dX3[:, 0].rearrange("p a b c -> p (a b c)"), actX)
            scalar.wait_ge(dve, 6)
            hats(wY3[:, 0].rearrange("p a b c -> p (a b c)"),
                 dY3[:, 0].rearrange("p a b c -> p (a b c)"), actY)
            for tr in (1, 2):
                scalar.wait_ge(gp, tr)
                hats(wX3[:, tr].rearrange("p a b c -> p (a b c)"),
                     dX3[:, tr].rearrange("p a b c -> p (a b c)"), actX)
            for tr in (1, 2):
                scalar.wait_ge(gp, 2 + tr)
                hats(wY3[:, tr].rearrange("p a b c -> p (a b c)"),
                     dY3[:, tr].rearrange("p a b c -> p (a b c)"), actY)

        @block.vector
        def _(vector):
            nc.vector.memset(zt[:], 0.0).then_inc(dve, 1)
            vector.wait_ge(sIN, 48)
            # delta[p,kx,s,j] = wtab[p,s] - txw[p, kx*TXW + s + j]
            nc.vector.tensor_tensor(
                delta[:],
                pap(wtab_sb[:], 0, [[0, 3], [1, S], [0, AW]]),
                pap(txw_sb[:], 0, [[TXW, 3], [1, S], [1, AW]]),
                Alu.subtract).then_inc(dve, 1)
            vector.wait_ge(sOF, 16)
            nc.vector.tensor_scalar(
                ox_cl[:], pap(off_sb[:], S, [[2 * S, K], [1, S]]),
                CLAMP, -CLAMP, Alu.min, Alu.max).then_inc(dve, 1)
            nc.vector.tensor_scalar(
                oy_cl[:], pap(off_sb[:], 0, [[2 * S, K], [1, S]]),
                CLAMP, -CLAMP, Alu.min, Alu.max).then_inc(dve, 1)
            # triple-0 weight grids on DVE (critical path)
            nc.vector.tensor_tensor(
                dX3[:, 0],
                pap(ox_cl[:], 0, [[S, 3], [1, S], [0, AW]]),
                pap(delta[:], 0, [[S * AW, 3], [AW, S], [1, AW]]),
                Alu.add).then_inc(dve, 1)
            nc.vector.tensor_tensor(
                dY3[:, 0],
                pap(oy_cl[:], 0, [[1, S], [0, AW], [S, 3]]),
                pap(iotay_sb[:], 0, [[0, S], [1, AW], [0, 3]]),
                Alu.subtract).then_inc(dve, 1)
            for tr in range(3):
                if tr == 0:
                    vector.wait_ge(sD, 64)
                vector.wait_ge(actX, tr + 1)
                for t in range(3):
                    wxb = (wX3[:, tr, t].unsqueeze(2)
                           .broadcast_to([P, S, AW, AW]))
                    ska = pap(strip[:], 96 * tr + t,
                              [[1, S], [96, AW], [1, AW]])
                    nc.vector.tensor_tensor(
                        pap(prod[:], t * S * AW * AW,
                            [[AW * AW, S], [AW, AW], [1, AW]]),
                        wxb, ska, Alu.mult)
                for t in range(3):
                    nc.vector.tensor_add(
                        t4[:, t],
                        pap(prod[:], t * S * AW * AW,
                            [[81, S], [9, AW], [1, 4]]),
                        pap(prod[:], t * S * AW * AW + 4,
                            [[81, S], [9, AW], [1, 4]]))
                for t in range(3):
                    nc.vector.tensor_add(
                        t2[:, t],
                        pap(t4[:], t * S * AW * 4, [[36, S], [4, AW], [1, 2]]),
                        pap(t4[:], t * S * AW * 4 + 2,
                            [[36, S], [4, AW], [1, 2]]))
                nc.vector.tensor_add(
                    t1f[:],
                    pap(t2[:], 0, [[1296, 3], [18, S], [2, AW]]),
                    pap(t2[:], 1, [[1296, 3], [18, S], [2, AW]]))
                nc.vector.tensor_add(
                    pap(colredA[:], tr * S * AW * 3,
                        [[27, S], [3, AW], [1, 3]]),
                    pap(t1f[:], 0, [[9, S], [1, AW], [648, 3]]),
                    pap(prod[:], 8, [[81, S], [9, AW], [5832, 3]])
                ).then_inc(dve, 1)
            # y-stage for triple 2 on DVE
            vector.wait_ge(actY, 3)
            nc.vector.tensor_mul(
                redV[:].rearrange("p a b c -> p (a b c)"),
                colredA[:, 2],
                wY3[:, 2].rearrange("p a b c -> p (a b c)"))
            nc.vector.tensor_add(
                y4V[:],
                pap(redV[:], 0, [[27, S], [3, 4], [1, 3]]),
                pap(redV[:], 12, [[27, S], [3, 4], [1, 3]]))
            nc.vector.tensor_add(
                y2V[:],
                pap(y4V[:], 0, [[12, S], [3, 2], [1, 3]]),
                pap(y4V[:], 6, [[12, S], [3, 2], [1, 3]]))
            nc.vector.tensor_add(
                y1V[:],
                pap(y2V[:], 0, [[6, S], [1, 3]]),
                pap(y2V[:], 3, [[6, S], [1, 3]]))
            nc.vector.tensor_add(
                res[:, 2], y1V[:],
                pap(redV[:], 24, [[27, S], [1, 3]])).then_inc(dve, 1)

    return nc


def _bf16_dtype():
    import ml_dtypes
    return ml_dtypes.bfloat16


def _tables():
    import ml_dtypes
    p = np.arange(P)[:, None]
    wtab = ((S * p + np.arange(S)[None, :]) % 96).astype(np.float16)
    base = np.empty((P, 3 * TXW), dtype=np.float16)
    for kx in range(3):
        i = np.arange(TXW)[None, :]
        base[:, kx * TXW:(kx + 1) * TXW] = (
            ((S * p + kx + i - 5) % 96) - (kx - 1)).astype(np.float16)
    iotay = np.tile(np.arange(AW, dtype=np.float16) - 4.0, (P, 1))
    ones = np.full((C, 2), 1.0 / C, dtype=ml_dtypes.bfloat16)
    return wtab, base, iotay, ones


def _get_nc():
    if "nc" not in _cached:
        _cached["nc"] = _build_nc()
    return _cached["nc"]


def _run(x, offset, trace=False):
    from concourse.bass_utils import run_bass_kernel_spmd

    nc = _get_nc()
    wtab, txw, iotay, ones = _tables()

    in_maps = []
    for b in range(B):
        in_maps.append({
            "x": np.ascontiguousarray(x[b].reshape(C, HW)).astype(
                _bf16_dtype()),
            "offset": np.ascontiguousarray(
                offset[b].reshape(2 * K, P, S).swapaxes(0, 1)
                .reshape(P, 2 * K * S)).astype(np.float16),
            "wtab": wtab,
            "txw": txw,
            "iotay": iotay,
            "ones": ones,
        })

    return run_bass_kernel_spmd(nc, in_maps, list(range(B)), trace=trace)


def kernel(x: np.ndarray, offset: np.ndarray, weight: np.ndarray) -> np.ndarray:
    results = _run(x, offset).results

    # host epilogue: replicate over t with per-(t,k) channel-sum scaling
    s = weight.reshape(C, T * K).sum(axis=0).astype(np.float32)  # [T*K]
    out = np.empty((B, T * K, H, W), dtype=np.float32)
    for b in range(B):
        # device layout: [P, tr, S, t] with k = 3*tr + t
        samp = (results[b]["out"].astype(np.float32)
                .reshape(P, 3, S, 3).transpose(1, 3, 0, 2)
                .reshape(K, H, W))
        for t in range(T):
            out[b, t * K:(t + 1) * K] = s[t * K:(t + 1) * K, None, None] * samp
    return out


# revision 14
# speedup vs baseline: 1.0175x; 1.0175x over previous
"""Deformable-correlation-fixed-weight kernel for 8 TRN2 NeuronCores.

Math: out[b, t*K+k, h, w] = sum_c samp[b,c,k,h,w] * weight[c,t,k].
With weight constant along c (DefCorFixW: weight = 1/C), this equals
s[t,k] * bilinear(mean_c x[b], py[b,k], px[b,k]); the device computes
the channel-mean image and the 9 bilinear-sampled maps per batch; the
host replicates over t and scales by s[t,k] = sum_c weight[c,t,k].

Sharding: data-parallel over batch B=8 across the 8 cores.

v6 design (DVE-centric, overlap-tuned):
  Pixels n = h*96+w are laid out n = 72*p + i (p partition, i slot).
  Offsets clamped to +-3.9995 => a 9x9 hat window suffices.
  Taps k = 3*ky + kx processed as 3 triples of constant ky=tr.

  All tensor math on DVE (GPSIMD compute shares the DVE SBUF port and
  poisons both engines - v5 lesson). ACT does the PSUM mean drain and
  the Y hat evaluations; the X hats run on DVE as 4x-mode
  tensor_scalar pairs. All six dX/dY weight-grid builds execute in
  the pre-strip window (tables arrive on the scalar queue first, so
  DVE starts at ~9us). Mean matmuls chase 2/3-chunk x DMAs; impad in
  6 sixths; strip in 4 gated quarter DMAs; a dummy ACT op preloads
  the PWP tables so the copy chain starts unstalled. Output bf16 in
  3 per-triple DMAs.
"""

import numpy as np

B, C, H, W = 8, 128, 96, 96
K = 9
T = 9
HW = H * W
P = 128          # partitions
S = HW // P      # 72 pixels per partition
AW = 9           # window side (rows and cols)
CLAMP = 3.9995
PADR = 8         # zero rows above/below in the flat padded image
NPAD = (H + 2 * PADR) * W          # 10752
STRIPLEN = 1042                    # per-partition strip (flat span)
STRIPOFF = 283                     # 72p - 485 + PADR*96
NCH = 512
NCHUNK = HW // NCH                 # 18
TXW = S + 12                       # txw row stride (kx*(S+12) + s + j)
# x chunk groups and their DMA queue (0 sync, 1 scalar, 2 gpsimd)
XG = ((0, 2, 0), (2, 2, 1), (4, 2, 2), (6, 3, 0), (9, 3, 1), (12, 3, 2),
      (15, 3, 0))
# impad sixth s may be written after ACT copy SIXTH_GATE[s] (1-based)
SIXTH_GATE = (2, 3, 5, 6, 8, 9)
# strip quarter q needs sC >= STRIP_GATE[q] (zt,zb,s0..s5 in order, 16 each)
STRIP_GATE = (64, 96, 112, 128)

_cached = {}


def _build_nc():
    import concourse.bass as bass
    import concourse.mybir as mybir
    from contextlib import ExitStack

    f32 = mybir.dt.float32
    bf16 = mybir.dt.bfloat16
    fp16 = mybir.dt.float16
    Alu = mybir.AluOpType
    Act = mybir.ActivationFunctionType

    nc = bass.Bass(detect_race_conditions=False)

    x_ext = nc.declare_dram_parameter("x", [C, HW], bf16, isOutput=False)
    off_ext = nc.declare_dram_parameter("offset", [P, 2 * K * S], fp16,
                                        isOutput=False)
    wtab_ext = nc.declare_dram_parameter("wtab", [P, S], fp16, isOutput=False)
    txw_ext = nc.declare_dram_parameter("txw", [P, 3 * TXW], fp16,
                                        isOutput=False)
    iotay_ext = nc.declare_dram_parameter("iotay", [P, AW], fp16,
                                          isOutput=False)
    ones_ext = nc.declare_dram_parameter("ones", [C, 2], bf16, isOutput=False)
    out_ext = nc.declare_dram_parameter("out", [P, K * S], bf16, isOutput=True)

    impad = nc.dram_tensor("impad", [NPAD], bf16)

    with ExitStack() as ctx:
        xb = ctx.enter_context(nc.sbuf_tensor([C, HW], bf16))
        off_sb = ctx.enter_context(nc.sbuf_tensor([P, 2 * K, S], fp16))
        wtab_sb = ctx.enter_context(nc.sbuf_tensor([P, S], fp16))
        txw_sb = ctx.enter_context(nc.sbuf_tensor([P, 3 * TXW], fp16))
        iotay_sb = ctx.enter_context(nc.sbuf_tensor([P, AW], fp16))
        ones_sb = ctx.enter_context(nc.sbuf_tensor([C, 2], bf16))
        m_flat = ctx.enter_context(nc.sbuf_tensor([1, HW], bf16))
        zt = ctx.enter_context(nc.sbuf_tensor([1, PADR * W], bf16))
        strip = ctx.enter_context(nc.sbuf_tensor([P, STRIPLEN], bf16))
        ox_cl = ctx.enter_context(nc.sbuf_tensor([P, K, S], fp16))
        oy_cl = ctx.enter_context(nc.sbuf_tensor([P, K, S], fp16))
        delta = ctx.enter_context(nc.sbuf_tensor([P, 3, S, AW], fp16))
        # dX3: t-major per triple [tr, t, s, j]; dY3: t-innermost [tr, s, j, t]
        dX3 = ctx.enter_context(nc.sbuf_tensor([P, 3, 3, S, AW], fp16))
        dY3 = ctx.enter_context(nc.sbuf_tensor([P, 3, S, AW, 3], fp16))
        aT = ctx.enter_context(nc.sbuf_tensor([P, S * AW * 3], fp16))
        aTv = ctx.enter_context(nc.sbuf_tensor([P, S * AW * 3], fp16))
        wX3 = ctx.enter_context(nc.sbuf_tensor([P, 3, 3, S, AW], bf16))
        wY3 = ctx.enter_context(nc.sbuf_tensor([P, 3, S, AW, 3], bf16))
        prod = ctx.enter_context(nc.sbuf_tensor([P, 3, S, AW * AW], bf16))
        t4 = ctx.enter_context(nc.sbuf_tensor([P, 3, S, AW, 4], bf16))
        t2 = ctx.enter_context(nc.sbuf_tensor([P, 3, S, AW, 2], bf16))
        t1f = ctx.enter_context(nc.sbuf_tensor([P, 3 * S * AW], bf16))
        colredA = ctx.enter_context(nc.sbuf_tensor([P, 3, S * AW * 3], bf16))
        redV = ctx.enter_context(nc.sbuf_tensor([P, S, AW, 3], bf16))
        y4V = ctx.enter_context(nc.sbuf_tensor([P, S, 4, 3], bf16))
        y2V = ctx.enter_context(nc.sbuf_tensor([P, S, 2, 3], bf16))
        y1V = ctx.enter_context(nc.sbuf_tensor([P, S * 3], bf16))
        res = ctx.enter_context(nc.sbuf_tensor([P, 3, S * 3], bf16))
        psA = ctx.enter_context(nc.psum_tensor([2, 4096], f32))
        sIN = ctx.enter_context(nc.semaphore("sIN"))    # wtab+txw+iotay
        sI2 = ctx.enter_context(nc.semaphore("sI2"))    # ones
        sOF = ctx.enter_context(nc.semaphore("sOF"))    # offsets
        sC = ctx.enter_context(nc.semaphore("sC"))      # impad writes
        sD = ctx.enter_context(nc.semaphore("sD"))      # strip quarters
        sO = ctx.enter_context(nc.semaphore("sO"))      # out
        sXg = [ctx.enter_context(nc.semaphore(f"sXg{g}"))
               for g in range(len(XG))]
        pe = ctx.enter_context(nc.semaphore("pe"))      # matmuls
        actC = ctx.enter_context(nc.semaphore("actC"))  # mean copies
        actX = ctx.enter_context(nc.semaphore("actX"))  # wX3 triples 1,2
        actY = ctx.enter_context(nc.semaphore("actY"))  # wY3 triples
        dve = ctx.enter_context(nc.semaphore("dve"))
        block = ctx.enter_context(nc.Block())

        # dve milestones: 1 memset, 2 delta, 3 ox, 4 oy, 5 dX0, 6 dY0,
        # 7 dX1, 8 dY1, 9 dX2, 10 dY2, 11/13/15 colred tr0/1/2,
        # 12/14/16 y tr0/1/2
        def pap(t_ap, off, dims):
            return bass.AP(tensor=t_ap.tensor, offset=t_ap.offset + off,
                           ap=[list(t_ap.ap[0])] + dims)

        def xg_dma(eng, g):
            c0, n, _ = XG[g]
            eng.dma_start(
                out=xb[:, c0 * NCH:(c0 + n) * NCH],
                in_=x_ext[:, c0 * NCH:(c0 + n) * NCH]).then_inc(sXg[g], 16)

        @block.sync
        def _(sync):
            for g in range(len(XG)):
                if XG[g][2] == 0:
                    xg_dma(sync, g)
            sync.wait_ge(dve, 1)
            sync.dma_start(
                out=bass.AP(tensor=impad[:].tensor, offset=impad[:].offset,
                            ap=[[1, 1], [1, PADR * W]]),
                in_=zt[:]).then_inc(sC, 16)
            sync.dma_start(
                out=bass.AP(tensor=impad[:].tensor,
                            offset=impad[:].offset + NPAD - PADR * W,
                            ap=[[1, 1], [1, PADR * W]]),
                in_=zt[:]).then_inc(sC, 16)
            sixth = HW // 6
            for s6 in range(6):
                sync.wait_ge(actC, SIXTH_GATE[s6])
                sync.dma_start(
                    out=bass.AP(tensor=impad[:].tensor,
                                offset=impad[:].offset + PADR * W + s6 * sixth,
                                ap=[[1, 1], [1, sixth]]),
                    in_=m_flat[:, s6 * sixth:(s6 + 1) * sixth]).then_inc(sC, 16)
            for q, gate in ((0, STRIP_GATE[0]), (2, STRIP_GATE[2]),
                            (3, STRIP_GATE[3])):
                sync.wait_ge(sC, gate)
                sync.dma_start(
                    out=strip[32 * q:32 * (q + 1)],
                    in_=bass.AP(tensor=impad[:].tensor,
                                offset=impad[:].offset + STRIPOFF + 32 * q * S,
                                ap=[[S, 32], [1, STRIPLEN]])).then_inc(sD, 16)
            sync.wait_ge(dve, 12)
            sync.dma_start(out=out_ext[:, 0:3 * S],
                           in_=res[:, 0]).then_inc(sO, 16)
            sync.wait_ge(dve, 14)
            sync.dma_start(out=out_ext[:, 3 * S:6 * S],
                           in_=res[:, 1]).then_inc(sO, 16)
            sync.wait_ge(dve, 16)
            sync.dma_start(out=out_ext[:, 6 * S:],
                           in_=res[:, 2]).then_inc(sO, 16)

        @block.gpsimd
        def _(g):
            g.dma_start(
                out=off_sb[:].rearrange("p a b -> p (a b)"),
                in_=off_ext[:]).then_inc(sOF, 16)
            for i in range(len(XG)):
                if XG[i][2] == 2:
                    xg_dma(g, i)
            g.wait_ge(sC, STRIP_GATE[1])
            g.dma_start(
                out=strip[32:64],
                in_=bass.AP(tensor=impad[:].tensor,
                            offset=impad[:].offset + STRIPOFF + 32 * S,
                            ap=[[S, 32], [1, STRIPLEN]])).then_inc(sD, 16)

        @block.tensor
        def _(tensor):
            tensor.wait_ge(sI2, 16)   # ones
            bounds = [c0 for c0, _, _ in XG]
            for g in range(NCHUNK):
                if g in bounds:
                    tensor.wait_ge(sXg[bounds.index(g)], 16)
                if g >= 8:
                    tensor.wait_ge(actC, (g - 8) // 2 + 1)
                nc.tensor.matmul(
                    psA[:, (g % 8) * NCH:(g % 8 + 1) * NCH],
                    ones_sb[:],
                    xb[:, g * NCH:(g + 1) * NCH],
                    start=True, stop=True,
                ).then_inc(pe, 1)

        @block.scalar
        def _(scalar):
            scalar.dma_start(out=wtab_sb[:], in_=wtab_ext[:]).then_inc(sIN, 16)
            scalar.dma_start(out=txw_sb[:], in_=txw_ext[:]).then_inc(sIN, 16)
            scalar.dma_start(out=iotay_sb[:],
                             in_=iotay_ext[:]).then_inc(sIN, 16)
            scalar.dma_start(out=ones_sb[:], in_=ones_ext[:]).then_inc(sI2, 16)
            for g in range(len(XG)):
                if XG[g][2] == 1:
                    xg_dma(scalar, g)
            # warm the PWP activation tables before the copy chain
            nc.scalar.activation(aT[0:1, 0:2], aT[0:1, 0:2], Act.Copy)
            nc.scalar.activation(aT[0:1, 0:2], aT[0:1, 0:2], Act.Abs)
            nc.scalar.activation(aT[0:1, 0:2], aT[0:1, 0:2], Act.Relu)
            for c in range(9):
                scalar.wait_ge(pe, 2 * c + 2)
                nc.scalar.activation(
                    m_flat[:, c * 1024:(c + 1) * 1024],
                    psA[0:1, (2 * c % 8) * NCH:(2 * c % 8 + 2) * NCH],
                    Act.Copy,
                ).then_inc(actC, 1)

            def hats(w_out, d_in, inc_sem):
                nc.scalar.activation(aT[:], d_in, Act.Abs)
                nc.scalar.activation(w_out, aT[:], Act.Relu,
                                     bias=1.0, scale=-1.0).then_inc(inc_sem, 1)

            scalar.wait_ge(dve, 5)
            hats(wX3[:, 0].rearrange("p a b c -> p (a b c)"),
                 dX3[:, 0].rearrange("p a b c -> p (a b c)"), actX)
            scalar.wait_ge(dve, 6)
            hats(wY3[:, 0].rearrange("p a b c -> p (a b c)"),
                 dY3[:, 0].rearrange("p a b c -> p (a b c)"), actY)
            scalar.wait_ge(dve, 7)
            hats(wX3[:, 1].rearrange("p a b c -> p (a b c)"),
                 dX3[:, 1].rearrange("p a b c -> p (a b c)"), actX)
            scalar.wait_ge(dve, 8)
            hats(wY3[:, 1].rearrange("p a b c -> p (a b c)"),
                 dY3[:, 1].rearrange("p a b c -> p (a b c)"), actY)
            scalar.wait_ge(dve, 9)
            hats(wX3[:, 2].rearrange("p a b c -> p (a b c)"),
                 dX3[:, 2].rearrange("p a b c -> p (a b c)"), actX)
            scalar.wait_ge(dve, 10)
            hats(wY3[:, 2].rearrange("p a b c -> p (a b c)"),
                 dY3[:, 2].rearrange("p a b c -> p (a b c)"), actY)

        @block.vector
        def _(vector):
            nc.vector.memset(zt[:], 0.0).then_inc(dve, 1)
            vector.wait_ge(sIN, 48)
            # delta[p,kx,s,j] = wtab[p,s] - txw[p, kx*TXW + s + j]
            nc.vector.tensor_tensor(
                delta[:],
                pap(wtab_sb[:], 0, [[0, 3], [1, S], [0, AW]]),
                pap(txw_sb[:], 0, [[TXW, 3], [1, S], [1, AW]]),
                Alu.subtract).then_inc(dve, 1)
            vector.wait_ge(sOF, 16)
            nc.vector.tensor_scalar(
                ox_cl[:], pap(off_sb[:], S, [[2 * S, K], [1, S]]),
                CLAMP, -CLAMP, Alu.min, Alu.max).then_inc(dve, 1)
            nc.vector.tensor_scalar(
                oy_cl[:], pap(off_sb[:], 0, [[2 * S, K], [1, S]]),
                CLAMP, -CLAMP, Alu.min, Alu.max).then_inc(dve, 1)
            for tr in range(3):
                # dX3[p,tr,t,s,j] = ox_cl[p,3tr+t,s] + delta[p,t,s,j]
                nc.vector.tensor_tensor(
                    dX3[:, tr],
                    pap(ox_cl[:], 3 * tr * S, [[S, 3], [1, S], [0, AW]]),
                    pap(delta[:], 0, [[S * AW, 3], [AW, S], [1, AW]]),
                    Alu.add).then_inc(dve, 1)
                # dY3[p,tr,s,j,t] = oy_cl[p,3tr+t,s] - iotay[p,j]
                nc.vector.tensor_tensor(
                    dY3[:, tr],
                    pap(oy_cl[:], 3 * tr * S, [[1, S], [0, AW], [S, 3]]),
                    pap(iotay_sb[:], 0, [[0, S], [1, AW], [0, 3]]),
                    Alu.subtract).then_inc(dve, 1)
            for tr in range(3):
                if tr == 0:
                    vector.wait_ge(sD, 64)
                vector.wait_ge(actX, tr + 1)
                for t in range(3):
                    wxb = (wX3[:, tr, t].unsqueeze(2)
                           .broadcast_to([P, S, AW, AW]))
                    ska = pap(strip[:], 96 * tr + t,
                              [[1, S], [96, AW], [1, AW]])
                    nc.vector.tensor_tensor(
                        pap(prod[:], t * S * AW * AW,
                            [[AW * AW, S], [AW, AW], [1, AW]]),
                        wxb, ska, Alu.mult)
                for t in range(3):
                    nc.vector.tensor_add(
                        t4[:, t],
                        pap(prod[:], t * S * AW * AW,
                            [[81, S], [9, AW], [1, 4]]),
                        pap(prod[:], t * S * AW * AW + 4,
                            [[81, S], [9, AW], [1, 4]]))
                for t in range(3):
                    nc.vector.tensor_add(
                        t2[:, t],
                        pap(t4[:], t * S * AW * 4, [[36, S], [4, AW], [1, 2]]),
                        pap(t4[:], t * S * AW * 4 + 2,
                            [[36, S], [4, AW], [1, 2]]))
                nc.vector.tensor_add(
                    t1f[:],
                    pap(t2[:], 0, [[1296, 3], [18, S], [2, AW]]),
                    pap(t2[:], 1, [[1296, 3], [18, S], [2, AW]]))
                nc.vector.tensor_add(
                    pap(colredA[:], tr * S * AW * 3,
                        [[27, S], [3, AW], [1, 3]]),
                    pap(t1f[:], 0, [[9, S], [1, AW], [648, 3]]),
                    pap(prod[:], 8, [[81, S], [9, AW], [5832, 3]])
                ).then_inc(dve, 1)
                # y-stage for this triple
                vector.wait_ge(actY, tr + 1)
                nc.vector.tensor_mul(
                    redV[:].rearrange("p a b c -> p (a b c)"),
                    colredA[:, tr],
                    wY3[:, tr].rearrange("p a b c -> p (a b c)"))
                nc.vector.tensor_add(
                    y4V[:],
                    pap(redV[:], 0, [[27, S], [3, 4], [1, 3]]),
                    pap(redV[:], 12, [[27, S], [3, 4], [1, 3]]))
                nc.vector.tensor_add(
                    y2V[:],
                    pap(y4V[:], 0, [[12, S], [3, 2], [1, 3]]),
                    pap(y4V[:], 6, [[12, S], [3, 2], [1, 3]]))
                nc.vector.tensor_add(
                    y1V[:],
                    pap(y2V[:], 0, [[6, S], [1, 3]]),
                    pap(y2V[:], 3, [[6, S], [1, 3]]))
                nc.vector.tensor_add(
                    res[:, tr], y1V[:],
                    pap(redV[:], 24, [[27, S], [1, 3]])).then_inc(dve, 1)

    return nc


def _bf16_dtype():
    import ml_dtypes
    return ml_dtypes.bfloat16


def _tables():
    import ml_dtypes
    p = np.arange(P)[:, None]
    wtab = ((S * p + np.arange(S)[None, :]) % 96).astype(np.float16)
    base = np.empty((P, 3 * TXW), dtype=np.float16)
    for kx in range(3):
        i = np.arange(TXW)[None, :]
        base[:, kx * TXW:(kx + 1) * TXW] = (
            ((S * p + kx + i - 5) % 96) - (kx - 1)).astype(np.float16)
    iotay = np.tile(np.arange(AW, dtype=np.float16) - 4.0, (P, 1))
    ones = np.full((C, 2), 1.0 / C, dtype=ml_dtypes.bfloat16)
    return wtab, base, iotay, ones


def _get_nc():
    if "nc" not in _cached:
        _cached["nc"] = _build_nc()
    return _cached["nc"]


def _run(x, offset, trace=False):
    from concourse.bass_utils import run_bass_kernel_spmd

    nc = _get_nc()
    wtab, txw, iotay, ones = _tables()

    in_maps = []
    for b in range(B):
        in_maps.append({
            "x": np.ascontiguousarray(x[b].reshape(C, HW)).astype(
                _bf16_dtype()),
            "offset": np.ascontiguousarray(
                offset[b].reshape(2 * K, P, S).swapaxes(0, 1)
                .reshape(P, 2 * K * S)).astype(np.float16),
            "wtab": wtab,
            "txw": txw,
            "iotay": iotay,
            "ones": ones,
        })

    return run_bass_kernel_spmd(nc, in_maps, list(range(B)), trace=trace)


def kernel(x: np.ndarray, offset: np.ndarray, weight: np.ndarray) -> np.ndarray:
    results = _run(x, offset).results

    # host epilogue: replicate over t with per-(t,k) channel-sum scaling
    s = weight.reshape(C, T * K).sum(axis=0).astype(np.float32)  # [T*K]
    out = np.empty((B, T * K, H, W), dtype=np.float32)
    for b in range(B):
        # device layout: [P, tr, S, t] with k = 3*tr + t
        samp = (results[b]["out"].astype(np.float32)
                .reshape(P, 3, S, 3).transpose(1, 3, 0, 2)
                .reshape(K, H, W))
        for t in range(T):
            out[b, t * K:(t + 1) * K] = s[t * K:(t + 1) * K, None, None] * samp
    return out


# revision 15
# speedup vs baseline: 1.2269x; 1.2058x over previous
"""Deformable-correlation-fixed-weight kernel for 8 TRN2 NeuronCores.

Math: out[b, t*K+k, h, w] = sum_c samp[b,c,k,h,w] * weight[c,t,k].
With weight constant along c (DefCorFixW: weight = 1/C), this equals
s[t,k] * bilinear(mean_c x[b], py[b,k], px[b,k]); the device computes
the channel-mean image and the 9 bilinear-sampled maps per batch; the
host replicates over t and scales by s[t,k] = sum_c weight[c,t,k].

Sharding: data-parallel over batch B=8 across the 8 cores.

v6 design (DVE-centric, overlap-tuned):
  Pixels n = h*96+w are laid out n = 72*p + i (p partition, i slot).
  Offsets clamped to +-3.9995 => a 9x9 hat window suffices.
  Taps k = 3*ky + kx processed as 3 triples of constant ky=tr.

  All tensor math on DVE (GPSIMD compute shares the DVE SBUF port and
  poisons both engines - v5 lesson). ACT does the PSUM mean drain and
  the Y hat evaluations; the X hats run on DVE as 4x-mode
  tensor_scalar pairs. All six dX/dY weight-grid builds execute in
  the pre-strip window (tables arrive on the scalar queue first, so
  DVE starts at ~9us). Mean matmuls chase 2/3-chunk x DMAs; impad in
  6 sixths; strip in 4 gated quarter DMAs; a dummy ACT op preloads
  the PWP tables so the copy chain starts unstalled. Output bf16 in
  3 per-triple DMAs.
"""

import numpy as np

B, C, H, W = 8, 128, 96, 96
K = 9
T = 9
HW = H * W
P = 128          # partitions
S = HW // P      # 72 pixels per partition
AW = 9           # window side (rows and cols)
CLAMP = 3.9995
PADR = 8         # zero rows above/below in the flat padded image
NPAD = (H + 2 * PADR) * W          # 10752
STRIPLEN = 1042                    # per-partition strip (flat span)
STRIPOFF = 283                     # 72p - 485 + PADR*96
NCH = 512
NCHUNK = HW // NCH                 # 18
TXW = S + 12                       # txw row stride (kx*(S+12) + s + j)
# x chunk groups and their DMA queue (0 sync, 1 scalar, 2 gpsimd)
XG = ((0, 2, 0), (2, 2, 1), (4, 2, 2), (6, 3, 0), (9, 3, 1), (12, 3, 2),
      (15, 3, 0))
# impad sixth s may be written after ACT copy SIXTH_GATE[s] (1-based)
SIXTH_GATE = (2, 3, 5, 6, 8, 9)
# strip quarter q needs sC >= STRIP_GATE[q] (zt,zb,s0..s5 in order, 16 each)
STRIP_GATE = (64, 96, 112, 128)

_cached = {}


def _build_nc():
    import concourse.bass as bass
    import concourse.mybir as mybir
    from contextlib import ExitStack

    f32 = mybir.dt.float32
    bf16 = mybir.dt.bfloat16
    fp16 = mybir.dt.float16
    Alu = mybir.AluOpType
    Act = mybir.ActivationFunctionType

    nc = bass.Bass(detect_race_conditions=False)

    x_ext = nc.declare_dram_parameter("x", [C, HW], bf16, isOutput=False)
    off_ext = nc.declare_dram_parameter("offset", [P, 2 * K * S], fp16,
                                        isOutput=False)
    wtab_ext = nc.declare_dram_parameter("wtab", [P, S], fp16, isOutput=False)
    txw_ext = nc.declare_dram_parameter("txw", [P, 3 * TXW], fp16,
                                        isOutput=False)
    iotay_ext = nc.declare_dram_parameter("iotay", [P, AW], fp16,
                                          isOutput=False)
    ones_ext = nc.declare_dram_parameter("ones", [C, 2], bf16, isOutput=False)
    out_ext = nc.declare_dram_parameter("out", [P, K * S], bf16, isOutput=True)

    impad = nc.dram_tensor("impad", [NPAD], bf16)

    with ExitStack() as ctx:
        xb = ctx.enter_context(nc.sbuf_tensor([C, HW], bf16))
        off_sb = ctx.enter_context(nc.sbuf_tensor([P, 2 * K, S], fp16))
        wtab_sb = ctx.enter_context(nc.sbuf_tensor([P, S], fp16))
        txw_sb = ctx.enter_context(nc.sbuf_tensor([P, 3 * TXW], fp16))
        iotay_sb = ctx.enter_context(nc.sbuf_tensor([P, AW], fp16))
        ones_sb = ctx.enter_context(nc.sbuf_tensor([C, 2], bf16))
        m_flat = ctx.enter_context(nc.sbuf_tensor([1, HW], bf16))
        zt = ctx.enter_context(nc.sbuf_tensor([1, PADR * W], bf16))
        strip = ctx.enter_context(nc.sbuf_tensor([P, STRIPLEN], bf16))
        ox_cl = ctx.enter_context(nc.sbuf_tensor([P, K, S], fp16))
        oy_cl = ctx.enter_context(nc.sbuf_tensor([P, K, S], fp16))
        delta = ctx.enter_context(nc.sbuf_tensor([P, 3, S, AW], fp16))
        # dX3: t-major per triple [tr, t, s, j]; dY3: t-innermost [tr, s, j, t]
        dX3 = ctx.enter_context(nc.sbuf_tensor([P, 3, 3, S, AW], fp16))
        dY3 = ctx.enter_context(nc.sbuf_tensor([P, 3, S, AW, 3], fp16))
        aT = ctx.enter_context(nc.sbuf_tensor([P, S * AW * 3], fp16))
        aTv = ctx.enter_context(nc.sbuf_tensor([P, S * AW * 3], fp16))
        wX3 = ctx.enter_context(nc.sbuf_tensor([P, 3, 3, S, AW], bf16))
        wY3 = ctx.enter_context(nc.sbuf_tensor([P, 3, S, AW, 3], bf16))
        prod = ctx.enter_context(nc.sbuf_tensor([P, 3, S, AW * AW], bf16))
        t4 = ctx.enter_context(nc.sbuf_tensor([P, 3, S, AW, 4], bf16))
        t2 = ctx.enter_context(nc.sbuf_tensor([P, 3, S, AW, 2], bf16))
        t1f = ctx.enter_context(nc.sbuf_tensor([P, 3 * S * AW], bf16))
        colredA = ctx.enter_context(nc.sbuf_tensor([P, 3, S * AW * 3], bf16))
        redV = ctx.enter_context(nc.sbuf_tensor([P, S, AW, 3], bf16))
        y4V = ctx.enter_context(nc.sbuf_tensor([P, S, 4, 3], bf16))
        y2V = ctx.enter_context(nc.sbuf_tensor([P, S, 2, 3], bf16))
        y1V = ctx.enter_context(nc.sbuf_tensor([P, S * 3], bf16))
        res = ctx.enter_context(nc.sbuf_tensor([P, 3, S * 3], bf16))
        psA = ctx.enter_context(nc.psum_tensor([2, 4096], f32))
        sIN = ctx.enter_context(nc.semaphore("sIN"))    # wtab+txw+iotay
        sI2 = ctx.enter_context(nc.semaphore("sI2"))    # ones
        sOF = ctx.enter_context(nc.semaphore("sOF"))    # offsets
        sC = ctx.enter_context(nc.semaphore("sC"))      # impad writes
        sD = ctx.enter_context(nc.semaphore("sD"))      # strip quarters
        sO = ctx.enter_context(nc.semaphore("sO"))      # out
        sXg = [ctx.enter_context(nc.semaphore(f"sXg{g}"))
               for g in range(len(XG))]
        pe = ctx.enter_context(nc.semaphore("pe"))      # matmuls
        actC = ctx.enter_context(nc.semaphore("actC"))  # mean copies
        actX = ctx.enter_context(nc.semaphore("actX"))  # wX3 triples 1,2
        actY = ctx.enter_context(nc.semaphore("actY"))  # wY3 triples
        dve = ctx.enter_context(nc.semaphore("dve"))
        block = ctx.enter_context(nc.Block())

        # dve milestones: 1 memset, 2 delta, 3 ox, 4 oy, 5 dX0, 6 dY0,
        # 7 dX1, 8 dY1, 9 dX2, 10 dY2, 11/13/15 colred tr0/1/2,
        # 12/14/16 y tr0/1/2
        def pap(t_ap, off, dims):
            return bass.AP(tensor=t_ap.tensor, offset=t_ap.offset + off,
                           ap=[list(t_ap.ap[0])] + dims)

        def xg_dma(eng, g):
            c0, n, _ = XG[g]
            eng.dma_start(
                out=xb[:, c0 * NCH:(c0 + n) * NCH],
                in_=x_ext[:, c0 * NCH:(c0 + n) * NCH]).then_inc(sXg[g], 16)

        @block.sync
        def _(sync):
            sync.dma_start(out=wtab_sb[:], in_=wtab_ext[:]).then_inc(sIN, 16)
            sync.dma_start(out=txw_sb[:], in_=txw_ext[:]).then_inc(sIN, 16)
            sync.dma_start(out=iotay_sb[:],
                           in_=iotay_ext[:]).then_inc(sIN, 16)
            sync.dma_start(out=ones_sb[:], in_=ones_ext[:]).then_inc(sI2, 16)
            for g in range(len(XG)):
                if XG[g][2] == 0:
                    xg_dma(sync, g)
            sync.wait_ge(dve, 1)
            sync.dma_start(
                out=bass.AP(tensor=impad[:].tensor, offset=impad[:].offset,
                            ap=[[1, 1], [1, PADR * W]]),
                in_=zt[:]).then_inc(sC, 16)
            sync.dma_start(
                out=bass.AP(tensor=impad[:].tensor,
                            offset=impad[:].offset + NPAD - PADR * W,
                            ap=[[1, 1], [1, PADR * W]]),
                in_=zt[:]).then_inc(sC, 16)
            sixth = HW // 6
            for s6 in range(6):
                sync.wait_ge(actC, SIXTH_GATE[s6])
                sync.dma_start(
                    out=bass.AP(tensor=impad[:].tensor,
                                offset=impad[:].offset + PADR * W + s6 * sixth,
                                ap=[[1, 1], [1, sixth]]),
                    in_=m_flat[:, s6 * sixth:(s6 + 1) * sixth]).then_inc(sC, 16)
            for q, gate in ((0, STRIP_GATE[0]), (2, STRIP_GATE[2]),
                            (3, STRIP_GATE[3])):
                sync.wait_ge(sC, gate)
                sync.dma_start(
                    out=strip[32 * q:32 * (q + 1)],
                    in_=bass.AP(tensor=impad[:].tensor,
                                offset=impad[:].offset + STRIPOFF + 32 * q * S,
                                ap=[[S, 32], [1, STRIPLEN]])).then_inc(sD, 16)
            sync.wait_ge(dve, 12)
            sync.dma_start(out=out_ext[:, 0:3 * S],
                           in_=res[:, 0]).then_inc(sO, 16)
            sync.wait_ge(dve, 14)
            sync.dma_start(out=out_ext[:, 3 * S:6 * S],
                           in_=res[:, 1]).then_inc(sO, 16)
            sync.wait_ge(dve, 16)
            sync.dma_start(out=out_ext[:, 6 * S:],
                           in_=res[:, 2]).then_inc(sO, 16)

        @block.gpsimd
        def _(g):
            g.dma_start(
                out=off_sb[:].rearrange("p a b -> p (a b)"),
                in_=off_ext[:]).then_inc(sOF, 16)
            for i in range(len(XG)):
                if XG[i][2] == 2:
                    xg_dma(g, i)
            g.wait_ge(sC, STRIP_GATE[1])
            g.dma_start(
                out=strip[32:64],
                in_=bass.AP(tensor=impad[:].tensor,
                            offset=impad[:].offset + STRIPOFF + 32 * S,
                            ap=[[S, 32], [1, STRIPLEN]])).then_inc(sD, 16)

        @block.tensor
        def _(tensor):
            tensor.wait_ge(sI2, 16)   # ones
            bounds = [c0 for c0, _, _ in XG]
            for g in range(NCHUNK):
                if g in bounds:
                    tensor.wait_ge(sXg[bounds.index(g)], 16)
                if g >= 8:
                    tensor.wait_ge(actC, (g - 8) // 2 + 1)
                nc.tensor.matmul(
                    psA[:, (g % 8) * NCH:(g % 8 + 1) * NCH],
                    ones_sb[:],
                    xb[:, g * NCH:(g + 1) * NCH],
                    start=True, stop=True,
                ).then_inc(pe, 1)

        @block.scalar
        def _(scalar):
            for g in range(len(XG)):
                if XG[g][2] == 1:
                    xg_dma(scalar, g)
            # warm the PWP activation tables before the copy chain
            nc.scalar.activation(aT[0:1, 0:2], aT[0:1, 0:2], Act.Copy)
            nc.scalar.activation(aT[0:1, 0:2], aT[0:1, 0:2], Act.Abs)
            nc.scalar.activation(aT[0:1, 0:2], aT[0:1, 0:2], Act.Relu)
            for c in range(9):
                scalar.wait_ge(pe, 2 * c + 2)
                nc.scalar.activation(
                    m_flat[:, c * 1024:(c + 1) * 1024],
                    psA[0:1, (2 * c % 8) * NCH:(2 * c % 8 + 2) * NCH],
                    Act.Copy,
                ).then_inc(actC, 1)

            def hats(w_out, d_in, inc_sem):
                nc.scalar.activation(aT[:], d_in, Act.Abs)
                nc.scalar.activation(w_out, aT[:], Act.Relu,
                                     bias=1.0, scale=-1.0).then_inc(inc_sem, 1)

            scalar.wait_ge(dve, 5)
            hats(wX3[:, 0].rearrange("p a b c -> p (a b c)"),
                 dX3[:, 0].rearrange("p a b c -> p (a b c)"), actX)
            scalar.wait_ge(dve, 6)
            hats(wY3[:, 0].rearrange("p a b c -> p (a b c)"),
                 dY3[:, 0].rearrange("p a b c -> p (a b c)"), actY)
            scalar.wait_ge(dve, 7)
            hats(wX3[:, 1].rearrange("p a b c -> p (a b c)"),
                 dX3[:, 1].rearrange("p a b c -> p (a b c)"), actX)
            scalar.wait_ge(dve, 8)
            hats(wY3[:, 1].rearrange("p a b c -> p (a b c)"),
                 dY3[:, 1].rearrange("p a b c -> p (a b c)"), actY)
            scalar.wait_ge(dve, 9)
            hats(wX3[:, 2].rearrange("p a b c -> p (a b c)"),
                 dX3[:, 2].rearrange("p a b c -> p (a b c)"), actX)
            scalar.wait_ge(dve, 10)
            hats(wY3[:, 2].rearrange("p a b c -> p (a b c)"),
                 dY3[:, 2].rearrange("p a b c -> p (a b c)"), actY)

        @block.vector
        def _(vector):
            nc.vector.memset(zt[:], 0.0).then_inc(dve, 1)
            vector.wait_ge(sIN, 48)
            # delta[p,kx,s,j] = wtab[p,s] - txw[p, kx*TXW + s + j]
            nc.vector.tensor_tensor(
                delta[:],
                pap(wtab_sb[:], 0, [[0, 3], [1, S], [0, AW]]),
                pap(txw_sb[:], 0, [[TXW, 3], [1, S], [1, AW]]),
                Alu.subtract).then_inc(dve, 1)
            vector.wait_ge(sOF, 16)
            nc.vector.tensor_scalar(
                ox_cl[:], pap(off_sb[:], S, [[2 * S, K], [1, S]]),
                CLAMP, -CLAMP, Alu.min, Alu.max).then_inc(dve, 1)
            nc.vector.tensor_scalar(
                oy_cl[:], pap(off_sb[:], 0, [[2 * S, K], [1, S]]),
                CLAMP, -CLAMP, Alu.min, Alu.max).then_inc(dve, 1)
            for tr in range(3):
                # dX3[p,tr,t,s,j] = ox_cl[p,3tr+t,s] + delta[p,t,s,j]
                nc.vector.tensor_tensor(
                    dX3[:, tr],
                    pap(ox_cl[:], 3 * tr * S, [[S, 3], [1, S], [0, AW]]),
                    pap(delta[:], 0, [[S * AW, 3], [AW, S], [1, AW]]),
                    Alu.add).then_inc(dve, 1)
                # dY3[p,tr,s,j,t] = oy_cl[p,3tr+t,s] - iotay[p,j]
                nc.vector.tensor_tensor(
                    dY3[:, tr],
                    pap(oy_cl[:], 3 * tr * S, [[1, S], [0, AW], [S, 3]]),
                    pap(iotay_sb[:], 0, [[0, S], [1, AW], [0, 3]]),
                    Alu.subtract).then_inc(dve, 1)
            for tr in range(3):
                if tr == 0:
                    vector.wait_ge(sD, 64)
                vector.wait_ge(actX, tr + 1)
                for t in range(3):
                    wxb = (wX3[:, tr, t].unsqueeze(2)
                           .broadcast_to([P, S, AW, AW]))
                    ska = pap(strip[:], 96 * tr + t,
                              [[1, S], [96, AW], [1, AW]])
                    nc.vector.tensor_tensor(
                        pap(prod[:], t * S * AW * AW,
                            [[AW * AW, S], [AW, AW], [1, AW]]),
                        wxb, ska, Alu.mult)
                for t in range(3):
                    nc.vector.tensor_add(
                        t4[:, t],
                        pap(prod[:], t * S * AW * AW,
                            [[81, S], [9, AW], [1, 4]]),
                        pap(prod[:], t * S * AW * AW + 4,
                            [[81, S], [9, AW], [1, 4]]))
                for t in range(3):
                    nc.vector.tensor_add(
                        t2[:, t],
                        pap(t4[:], t * S * AW * 4, [[36, S], [4, AW], [1, 2]]),
                        pap(t4[:], t * S * AW * 4 + 2,
                            [[36, S], [4, AW], [1, 2]]))
                nc.vector.tensor_add(
                    t1f[:],
                    pap(t2[:], 0, [[1296, 3], [18, S], [2, AW]]),
                    pap(t2[:], 1, [[1296, 3], [18, S], [2, AW]]))
                nc.vector.tensor_add(
                    pap(colredA[:], tr * S * AW * 3,
                        [[27, S], [3, AW], [1, 3]]),
                    pap(t1f[:], 0, [[9, S], [1, AW], [648, 3]]),
                    pap(prod[:], 8, [[81, S], [9, AW], [5832, 3]])
                ).then_inc(dve, 1)
                # y-stage for this triple
                vector.wait_ge(actY, tr + 1)
                nc.vector.tensor_mul(
                    redV[:].rearrange("p a b c -> p (a b c)"),
                    colredA[:, tr],
                    wY3[:, tr].rearrange("p a b c -> p (a b c)"))
                nc.vector.tensor_add(
                    y4V[:],
                    pap(redV[:], 0, [[27, S], [3, 4], [1, 3]]),
                    pap(redV[:], 12, [[27, S], [3, 4], [1, 3]]))
                nc.vector.tensor_add(
                    y2V[:],
                    pap(y4V[:], 0, [[12, S], [3, 2], [1, 3]]),
                    pap(y4V[:], 6, [[12, S], [3, 2], [1, 3]]))
                nc.vector.tensor_add(
                    y1V[:],
                    pap(y2V[:], 0, [[6, S], [1, 3]]),
                    pap(y2V[:], 3, [[6, S], [1, 3]]))
                nc.vector.tensor_add(
                    res[:, tr], y1V[:],
                    pap(redV[:], 24, [[27, S], [1, 3]])).then_inc(dve, 1)

    return nc


def _bf16_dtype():
    import ml_dtypes
    return ml_dtypes.bfloat16


def _tables():
    import ml_dtypes
    p = np.arange(P)[:, None]
    wtab = ((S * p + np.arange(S)[None, :]) % 96).astype(np.float16)
    base = np.empty((P, 3 * TXW), dtype=np.float16)
    for kx in range(3):
        i = np.arange(TXW)[None, :]
        base[:, kx * TXW:(kx + 1) * TXW] = (
            ((S * p + kx + i - 5) % 96) - (kx - 1)).astype(np.float16)
    iotay = np.tile(np.arange(AW, dtype=np.float16) - 4.0, (P, 1))
    ones = np.full((C, 2), 1.0 / C, dtype=ml_dtypes.bfloat16)
    return wtab, base, iotay, ones


def _get_nc():
    if "nc" not in _cached:
        _cached["nc"] = _build_nc()
    return _cached["nc"]


def _run(x, offset, trace=False):
    from concourse.bass_utils import run_bass_kernel_spmd

    nc = _get_nc()
    wtab, txw, iotay, ones = _tables()

    in_maps = []
    for b in range(B):
        in_maps.append({
            "x": np.ascontiguousarray(x[b].reshape(C, HW)).astype(
                _bf16_dtype()),
            "offset": np.ascontiguousarray(
                offset[b].reshape(2 * K, P, S).swapaxes(0, 1)
                .reshape(P, 2 * K * S)).astype(np.float16),
            "wtab": wtab,
            "txw": txw,
            "iotay": iotay,
            "ones": ones,
        })

    return run_bass_kernel_spmd(nc, in_maps, list(range(B)), trace=trace)


def kernel(x: np.ndarray, offset: np.ndarray, weight: np.ndarray) -> np.ndarray:
    results = _run(x, offset).results

    # host epilogue: replicate over t with per-(t,k) channel-sum scaling
    s = weight.reshape(C, T * K).sum(axis=0).astype(np.float32)  # [T*K]
    out = np.empty((B, T * K, H, W), dtype=np.float32)
    for b in range(B):
        # device layout: [P, tr, S, t] with k = 3*tr + t
        samp = (results[b]["out"].astype(np.float32)
                .reshape(P, 3, S, 3).transpose(1, 3, 0, 2)
                .reshape(K, H, W))
        for t in range(T):
            out[b, t * K:(t + 1) * K] = s[t * K:(t + 1) * K, None, None] * samp
    return out
